# revision 19
# baseline (speedup 1.0000x reference)
"""RWKV6 (nn_ExtendedMemory) Trainium2 kernel — 8 NeuronCores, token-split.

Sharding: core c -> batch c//2, token-half c%2 (tokens [h*T/2, (h+1)*T/2)),
ALL 16 heads / full channels per core. The WKV recurrence state crosses the
half boundary once per layer: each core runs its windows provisionally with
zero incoming state while storing the pre-GroupNorm WKV output, decayed r,
and gate g; a small pair AllGather then ships h=0's end state + last-token
residual, and a correction pass adds r@(S_in*Wcum) and redoes
GroupNorm->gate->Wo->residual. Token-shift boundaries cross via the same
AllGather (4KB); the tm0 boundary is raw x so the host supplies it.

On-chip layout ("P2"): channels on the partition axis, tokens on the free
axis. WKV uses chunked linear attention (chunk C=128) with intra-chunk
triangular matmuls and a cross-chunk [hs,hs] state per head, processed in
two head-groups of 8 to fit PSUM.
"""

import numpy as np
import ml_dtypes

import concourse.bass as bass
import concourse.mybir as mybir
import concourse.tile as tile
from concourse.bass_utils import run_bass_kernel_spmd

dt = mybir.dt
Alu = mybir.AluOpType
Act = mybir.ActivationFunctionType
BF16 = ml_dtypes.bfloat16

L, D, HS, E, ED, FE = 2, 1024, 64, 32, 64, 3
H = D // HS            # 16 heads, all on every core
B = 4
EPS = 1e-5
NH = H                 # heads per core
CH = NH * HS           # 1024 channels per core
DT = D // 128          # 8 D-tiles
CT = CH // 128         # 8 chan-tiles per core
FT = FE * D // 128     # 24 ffn-tiles per core
CK = 128               # wkv chunk
WIN = 256              # token window
NHG = 2                # head groups (8 heads each) per chunk for PSUM fit
JG = CT // NHG         # 4 chan-subtiles per head group

REPLICA_GROUPS = [[0, 1], [2, 3], [4, 5], [6, 7]]

TC = tile.TileContext

_wsplit_counter = [0]


def _split_sync_waits(nc, scratch=None, max_waits=1):
    """walrus in this container rejects >1 sync wait per instruction.

    For single-queue engines (PE/DVE/ACT/SP) excess waits move onto
    same-engine standalone EventSemaphore instructions placed immediately
    before the owner (engine streams are strict FIFO, so this is
    equivalent). GpSimd fans instructions across 8 Q7 queues, so a
    standalone wait there guards nothing — instead its waits are relayed:
    SP waits each semaphore (EVSEM chain), then bumps a scratch semaphore
    that the Pool instruction waits on (its single allowed wait)."""
    if scratch is None:
        scratch = nc.alloc_semaphore("wsplit_scratch")
    scratch_count = [0]

    def evsem(engine, waits, updates=()):
        _wsplit_counter[0] += 1
        ev = mybir.InstEventSemaphore(
            name=f"I-wsplit-{_wsplit_counter[0]}", ins=[], outs=[])
        ev.engine = engine
        ev.sync_info = mybir.SyncInfo(on_wait=list(waits),
                                      on_update=list(updates))
        return ev

    sp = mybir.EngineType.Activation
    for f in nc.m.functions:
        for bb in f.blocks:
            out = []
            changed = False
            for inst in bb.instructions:
                tname = type(inst).__name__
                si = inst.sync_info
                if si is not None and len(si.on_wait) > max_waits:
                    waits = list(si.on_wait)
                    changed = True
                    if (inst.engine == mybir.EngineType.Pool
                            or "Collective" in tname):
                        for wv in waits:
                            out.append(evsem(sp, [wv]))
                        scratch_count[0] += 1
                        out.append(evsem(sp, [], [mybir.SyncUpdate(
                            sync_type="semaphore", id=scratch.num,
                            update_mode="sem-inc", update_value=1)]))
                        keep = [mybir.SyncWait(
                            sync_type="semaphore", id=scratch.num,
                            wait_mode="sem-ge-imm",
                            wait_value=scratch_count[0])]
                    else:
                        extra, keep = waits[:-max_waits], waits[-max_waits:]
                        while extra:
                            chunk, extra = (extra[:max_waits],
                                            extra[max_waits:])
                            out.append(evsem(inst.engine, chunk))
                    inst.sync_info = mybir.SyncInfo(
                        on_wait=keep, on_update=list(si.on_update))
                out.append(inst)
            if changed:
                bb.instructions = out


def build(nc, TH, skip_gn_affine, skip_ln_w, split_waits=True):
    """TH = tokens per core (half the sequence)."""
    import os
    _STOP = int(os.environ.get("KSTOP", "99"))
    _SUB = int(os.environ.get("KSUB", "99"))
    W = min(WIN, TH)
    assert TH % W == 0
    NW = TH // W
    NCH = W // CK or 1
    assert W % CK == 0 or W < CK
    CKe = min(CK, W)
    NCHUNKS = NW * NCH      # chunks per core, global

    f32, bf = dt.float32, dt.bfloat16

    def din(name, shape, d=f32):
        return nc.dram_tensor(name, shape, d, kind="ExternalInput")

    x_d = din("x", [D, TH])
    y_d = nc.dram_tensor("y", [D, TH], f32, kind="ExternalOutput")

    Wr_d = din("Wr", [L, DT, 128, CH], bf)
    Wk_d = din("Wk", [L, DT, 128, CH], bf)
    Wv_d = din("Wv", [L, DT, 128, CH], bf)
    Wg_d = din("Wg", [L, DT, 128, CH], bf)
    tmw1_d = din("tm_w1", [L, DT, 128, 5 * E], bf)
    tmw2_d = din("tm_w2", [L, 2, 128, D], bf)      # f-pairs packed on partitions
    tdw1_d = din("td_w1", [L, DT, 128, ED], bf)
    tdw2_d = din("td_w2", [L, ED, CH], bf)
    Wo_d = din("Wo", [L, CT, 128, D], bf)
    cWk_d = din("cWk", [L, DT, 128, FT * 128], bf)
    cWv_d = din("cWv", [L, FT, 128, D], bf)
    cWr_d = din("cWr", [L, DT, 128, D], bf)

    ln1_d = din("ln1_w", [L, 128, DT])
    ln2_d = din("ln2_w", [L, 128, DT])
    lnf_d = din("ln_out_w", [128, DT])
    maas_d = {n: din(n, [L, 128, DT]) for n in
              ["x_maa", "w_maa", "k_maa", "v_maa", "r_maa", "g_maa",
               "ck_maa", "cr_maa"]}
    tdb_d = din("td_bias", [L, 128, CT])
    u_d = din("u_first", [L, 128, CT])
    gnw_d = din("gn_w", [L, CH])
    gnb_d = din("gn_b", [L, CH])
    ident_d = din("ident_c", [128, 128], bf)
    maskstr_d = din("maskstr_c", [128, 128])
    blkdiag_d = din("blkdiag_c", [128, 2], bf)
    xb0_d = din("xb0", [128, DT])      # raw-x boundary token (h=1), zeros h=0
    tmask_d = din("tmask", [1, CT])    # 0.0 on h=0 cores, 1.0 on h=1 cores

    def bcast_ap(dram_ap, parts=128):
        return bass.AP(tensor=dram_ap.tensor, offset=dram_ap.offset,
                       ap=[[0, parts]] + list(dram_ap.ap))

    # payload: S (CT*HS f32) + boundary resid (DT)
    PAYS = CT * HS
    PH = PAYS + DT

    wsplit_sem = nc.alloc_semaphore("wsplit_scratch")
    nc.sync.sem_clear(wsplit_sem)

    with TC(nc) as tc:
        import contextlib
        ctx = contextlib.ExitStack()
        with ctx:
            const = ctx.enter_context(tc.tile_pool(name="const", bufs=1))
            dram = ctx.enter_context(tc.tile_pool(name="dramb", bufs=1,
                                                  space="DRAM"))
            xrp = ctx.enter_context(tc.tile_pool(name="xrp", bufs=1,
                                                 space="DRAM"))
            xresA = xrp.tile([DT, 128, TH], f32, tag="xresA")
            xresB = xrp.tile([DT, 128, TH], f32, tag="xresB")

            def x_src(phase, j, sl):
                """DRAM source of the residual entering a given phase."""
                if phase == 0:
                    return x_d[128 * j:128 * (j + 1), sl]
                buf = xresB if phase % 2 == 1 else xresA
                return buf[j, :, sl]

            def x_dst(phase, j, sl):
                """DRAM residual written by a given phase."""
                buf = xresB if phase % 2 == 0 else xresA
                return buf[j, :, sl]

            ident = const.tile([128, 128], bf)
            nc.sync.dma_start(out=ident, in_=ident_d[:, :])
            maskstr = const.tile([128, 128], f32)   # keep j < i over [j, i]
            nc.sync.dma_start(out=maskstr, in_=maskstr_d[:, :])
            blkdiag = const.tile([128, 2], bf)      # col a = partitions 64a..
            nc.sync.dma_start(out=blkdiag, in_=blkdiag_d[:, :])
            ones_bf = const.tile([128, 1], bf)
            nc.vector.memset(ones_bf, 1.0)
            ones_f = const.tile([128, 1], f32)
            nc.vector.memset(ones_f, 1.0)
            ones_row = const.tile([1, 128], f32)
            nc.vector.memset(ones_row, 1.0)
            zerosCK = const.tile([128, CKe], f32)
            nc.vector.memset(zerosCK, 0.0)
            epst = const.tile([128, 1], f32)
            nc.vector.memset(epst, EPS)
            tmask_t = const.tile([128, 1], f32)
            nc.sync.dma_start(out=tmask_t, in_=bcast_ap(tmask_d[0, 0:1]))
            itmask_t = const.tile([128, 1], f32)
            nc.vector.memset(itmask_t, 1.0)
            nc.vector.tensor_sub(itmask_t, itmask_t, tmask_t)
            xb0_t = const.tile([128, DT], f32)
            nc.sync.dma_start(out=xb0_t, in_=xb0_d[:, :])

            def layer_norm(pool, ps_pool, lnw_t, xsrc_f32, Wn, out=None,
                           out_dt=bf, tag="ln"):
                """LN over channels. xsrc_f32: [128, DT, Wn] SBUF fp32.
                Returns xln [128, DT, Wn] out_dt (written to `out` if given)."""
                ps = ps_pool.tile([128, Wn], f32, tag="mm")
                ps_sq = ps_pool.tile([128, Wn], f32, tag="mm")
                for j in range(DT):
                    sqj = pool.tile([128, Wn], f32, tag=f"{tag}_sqj")
                    nc.vector.tensor_mul(sqj, xsrc_f32[:, j, :],
                                         xsrc_f32[:, j, :])
                    nc.tensor.matmul(ps[0:1, :], lhsT=ones_f,
                                     rhs=xsrc_f32[:, j, :],
                                     start=(j == 0), stop=(j == DT - 1))
                    nc.tensor.matmul(ps_sq[0:1, :], lhsT=ones_f, rhs=sqj,
                                     start=(j == 0), stop=(j == DT - 1))
                mu = pool.tile([1, Wn], f32, tag=f"{tag}_mu")
                nc.vector.tensor_scalar_mul(mu, ps[0:1, :], 1.0 / D)
                musq = pool.tile([1, Wn], f32, tag=f"{tag}_musq")
                nc.vector.tensor_mul(musq, mu, mu)
                var = pool.tile([1, Wn], f32, tag=f"{tag}_var")
                nc.vector.scalar_tensor_tensor(out=var, in0=ps_sq[0:1, :],
                                               scalar=1.0 / D, in1=musq,
                                               op0=Alu.mult, op1=Alu.subtract)
                sd = pool.tile([1, Wn], f32, tag=f"{tag}_sd")
                nc.scalar.activation(sd, var, Act.Sqrt, bias=epst[0:1],
                                     scale=1.0)
                rstd = pool.tile([1, Wn], f32, tag=f"{tag}_rstd")
                nc.vector.reciprocal(rstd, sd)
                ps_b = ps_pool.tile([128, Wn], f32, tag="mm")
                nc.tensor.matmul(ps_b, lhsT=ones_row, rhs=mu, start=True,
                                 stop=True)
                mur = pool.tile([128, Wn], f32, tag=f"{tag}_mur")
                nc.vector.tensor_copy(mur, ps_b)
                ps_b2 = ps_pool.tile([128, Wn], f32, tag="mm")
                nc.tensor.matmul(ps_b2, lhsT=ones_row, rhs=rstd, start=True,
                                 stop=True)
                rstdr = pool.tile([128, Wn], f32, tag=f"{tag}_rstdr")
                nc.vector.tensor_copy(rstdr, ps_b2)
                if out is None:
                    out = pool.tile([128, DT, Wn], out_dt, tag=f"{tag}_out",
                                    name=f"{tag}_out")
                xln = out
                tmp = pool.tile([128, Wn], f32, tag=f"{tag}_tmp")
                for j in range(DT):
                    nc.vector.tensor_sub(tmp, xsrc_f32[:, j, :], mur)
                    if skip_ln_w:
                        nc.vector.tensor_mul(xln[:, j, :], tmp, rstdr)
                    else:
                        nc.vector.scalar_tensor_tensor(
                            out=xln[:, j, :], in0=tmp,
                            scalar=lnw_t[:, j:j + 1],
                            in1=rstdr, op0=Alu.mult, op1=Alu.mult)
                return xln

            def token_shift(pool, xln, prev_last, Wn):
                """sx = shift(xln) - xln; prev_last: [128, DT, 1] (can be a
                slice of the previous window's xln or the boundary LN)."""
                sx = pool.tile([128, DT, Wn], bf, tag="sx")
                for j in range(DT):
                    nc.vector.tensor_sub(sx[:, j, 1:Wn], xln[:, j, 0:Wn - 1],
                                         xln[:, j, 1:Wn])
                    nc.vector.tensor_sub(sx[:, j, 0:1], prev_last[:, j, 0:1],
                                         xln[:, j, 0:1])
                return sx

            def mm_chain(ps, lhsT_f, rhs_f, nkt):
                for kt in range(nkt):
                    nc.tensor.matmul(ps, lhsT=lhsT_f(kt), rhs=rhs_f(kt),
                                     start=(kt == 0), stop=(kt == nkt - 1))

            def group_norm(pool, O_sb, gnw_r, gnb_r, hg):
                """GN per head over O_sb [128, 512] (8 heads). Returns Ogn."""
                stt = pool.tile([128, 8, 6], f32, tag="gn_st")
                mv = pool.tile([128, 8, 2], f32, tag="gn_mv")
                for hl in range(8):
                    nc.vector.bn_stats(stt[:, hl, :],
                                       O_sb[:, HS * hl:HS * (hl + 1)])
                    nc.vector.bn_aggr(mv[:, hl, :], stt[:, hl, :])
                sd8 = pool.tile([128, 8], f32, tag="gn_sd")
                nc.scalar.activation(
                    sd8.rearrange("p (c u) -> p c u", u=1),
                    mv[:, :, 1:2], Act.Sqrt, bias=epst, scale=1.0)
                rs8 = pool.tile([128, 8], f32, tag="gn_rs")
                nc.vector.reciprocal(rs8, sd8)
                nm8 = pool.tile([128, 8], f32, tag="gn_nm")
                nc.vector.tensor_mul(
                    nm8.rearrange("p (c u) -> p c u", u=1),
                    mv[:, :, 0:1],
                    rs8.rearrange("p (c u) -> p c u", u=1))
                nc.vector.tensor_scalar_mul(nm8, nm8, -1.0)
                Ogn = pool.tile([128, 512], bf, tag="Ogn")
                for hl in range(8):
                    nc.scalar.activation(
                        Ogn[:, HS * hl:HS * (hl + 1)],
                        O_sb[:, HS * hl:HS * (hl + 1)], Act.Identity,
                        bias=nm8[:, hl:hl + 1], scale=rs8[:, hl:hl + 1])
                if not skip_gn_affine:
                    t1g = pool.tile([128, 512], bf, tag="gn_t1")
                    nc.vector.tensor_mul(t1g, Ogn,
                                         gnw_r[:, 512 * hg:512 * (hg + 1)])
                    nc.vector.tensor_add(Ogn, t1g,
                                         gnb_r[:, 512 * hg:512 * (hg + 1)])
                return Ogn

            # spill buffers (DRAM), reused across layers
            opre_d = dram.tile([NW, NCH, NHG, 128, 512], bf, tag="opre")
            rt_d = dram.tile([NW, 128, CT, W], bf, tag="rt")
            g_d = dram.tile([NW, 128, CT, W], bf, tag="g")
            pay_t = dram.tile([128, PH], f32, tag="pay")
            payg_t = dram.tile([128, PH], f32, tag="payg")
            pay2_t = dram.tile([128, DT], f32, tag="pay2")
            payg2_t = dram.tile([128, DT], f32, tag="payg2")

            # boundary residual for the next phase, masked ([128, DT] f32)
            bres = const.tile([128, DT], f32)
            nc.vector.tensor_copy(bres, xb0_t)
            # decay running product / per-chunk prefix products; must outlive
            # the main-pass pools (read by the correction pass)
            Wrun = const.tile([128, CT], f32)
            Wcum = const.tile([128, CT, NCHUNKS], f32)

            # ================= layers =================
            NLAYERS = L if _STOP >= 4 else (1 if _STOP >= 0 else 0)
            for l in range(NLAYERS):
                phase_tm = 2 * l
                phase_cm = 2 * l + 1

                # ---------------- time mix: main (provisional) pass --------
                with tc.tile_pool(name=f"wtmA{l}", bufs=1) as wpA, \
                     tc.tile_pool(name=f"wtmB{l}", bufs=1) as wpB, \
                     tc.tile_pool(name=f"vec{l}", bufs=1) as vp, \
                     tc.tile_pool(name=f"tmx{l}", bufs=1) as px, \
                     tc.tile_pool(name=f"tml{l}", bufs=2) as pl, \
                     tc.tile_pool(name=f"tma{l}", bufs=1) as pw, \
                     tc.tile_pool(name=f"tmb{l}", bufs=1) as pk, \
                     tc.tile_pool(name=f"tmc{l}", bufs=2) as pb, \
                     tc.tile_pool(name=f"st{l}", bufs=2) as sp, \
                     tc.tile_pool(name=f"psa{l}", bufs=2, space="PSUM") as psA, \
                     tc.tile_pool(name=f"psb{l}", bufs=1, space="PSUM") as psB, \
                     tc.tile_pool(name=f"pst{l}", bufs=1, space="PSUM") as psT:

                    w_r = wpA.tile([128, DT, CH], bf, tag="w_r")
                    w_k = wpA.tile([128, DT, CH], bf, tag="w_k")
                    w_v = wpA.tile([128, DT, CH], bf, tag="w_v")
                    w_g = wpA.tile([128, DT, CH], bf, tag="w_g")
                    for j in range(DT):
                        nc.sync.dma_start(out=w_r[:, j, :], in_=Wr_d[l, j])
                        nc.sync.dma_start(out=w_k[:, j, :], in_=Wk_d[l, j])
                        nc.sync.dma_start(out=w_v[:, j, :], in_=Wv_d[l, j])
                        nc.sync.dma_start(out=w_g[:, j, :], in_=Wg_d[l, j])
                    tmw1 = wpA.tile([128, DT, 5 * E], bf, tag="tmw1")
                    tdw1 = wpA.tile([128, DT, ED], bf, tag="tdw1")
                    for j in range(DT):
                        nc.sync.dma_start(out=tmw1[:, j, :], in_=tmw1_d[l, j])
                        nc.sync.dma_start(out=tdw1[:, j, :], in_=tdw1_d[l, j])
                    tmw2 = wpA.tile([128, 2, D], bf, tag="tmw2")
                    for j in range(2):
                        nc.sync.dma_start(out=tmw2[:, j, :], in_=tmw2_d[l, j])
                    tdw2 = wpA.tile([ED, CH], bf, tag="tdw2")
                    nc.sync.dma_start(out=tdw2, in_=tdw2_d[l])
                    wo = wpB.tile([128, CT, D], bf, tag="wo")
                    for j in range(CT):
                        nc.sync.dma_start(out=wo[:, j, :], in_=Wo_d[l, j])

                    ln1w = vp.tile([128, DT], f32, tag="ln1w")
                    nc.sync.dma_start(out=ln1w, in_=ln1_d[l])
                    maat = {}
                    for n in ["x_maa", "w_maa", "k_maa", "v_maa", "r_maa",
                              "g_maa"]:
                        maat[n] = vp.tile([128, DT], f32, tag=n, name=n)
                        nc.sync.dma_start(out=maat[n], in_=maas_d[n][l])
                    tdb = vp.tile([128, CT], f32, tag="tdb")
                    nc.sync.dma_start(out=tdb, in_=tdb_d[l])
                    ut = vp.tile([128, CT], f32, tag="ut")
                    nc.sync.dma_start(out=ut, in_=u_d[l])
                    gnw_r = vp.tile([128, CH], bf, tag="gnw_r")
                    gnb_r = vp.tile([128, CH], bf, tag="gnb_r")
                    if not skip_gn_affine:
                        nc.sync.dma_start(out=gnw_r, in_=bcast_ap(gnw_d[l]))
                        nc.sync.dma_start(out=gnb_r, in_=bcast_ap(gnb_d[l]))

                    S_cur = sp.tile([128, CT, HS], bf, tag="S")
                    nc.vector.memset(S_cur, 0.0)
                    # Wrun: running decay product, seeded with tmask so the
                    # correction term vanishes on h=0 cores.
                    nc.sync.dma_start(out=Wrun, in_=bcast_ap(tmask_d[0]))

                    # boundary LN for the first window's token shift
                    bres_v = px.tile([128, DT, 1], f32, tag="bres_v")
                    nc.vector.tensor_copy(bres_v.rearrange("p a b -> p (a b)"),
                                          bres)
                    bln = layer_norm(px, psA, ln1w, bres_v, 1, tag="bln")

                    xln_prev = None
                    for w in range(NW):
                        sl = slice(w * W, (w + 1) * W)
                        xw = pw.tile([128, DT, W], f32, tag="xw")
                        for j in range(DT):
                            nc.sync.dma_start(out=xw[:, j, :],
                                              in_=x_src(phase_tm, j, sl))
                        xln = pl.tile([128, DT, W], bf, tag="xln")
                        layer_norm(pw, psA, ln1w, xw, W, out=xln)
                        prev_last = (bln if w == 0
                                     else xln_prev[:, :, W - 1:W])
                        sx = token_shift(pw, xln, prev_last, W)
                        xln_prev = xln

                        xxx = pw.tile([128, DT, W], bf, tag="xxx")
                        for j in range(DT):
                            nc.vector.scalar_tensor_tensor(
                                out=xxx[:, j, :], in0=sx[:, j, :],
                                scalar=maat["x_maa"][:, j:j + 1],
                                in1=xln[:, j, :], op0=Alu.mult, op1=Alu.add)

                        t320 = pw.tile([128, 2, W], bf, tag="t320")
                        for mt in range(2):
                            msz = min(128, 5 * E - mt * 128)
                            ps = psA.tile([128, W], f32, tag="mm")
                            mm_chain(ps[0:msz, :],
                                     lambda kt, mt=mt, msz=msz:
                                         tmw1[:, kt, mt * 128:mt * 128 + msz],
                                     lambda kt: xxx[:, kt, :], DT)
                            nc.scalar.activation(t320[0:msz, mt, :],
                                                 ps[0:msz, :], Act.Tanh)

                        # bname order: w last is NOT needed; order so each bx
                        # is consumed right after it is built (2-slot buffer)
                        bnames = ["k_maa", "v_maa", "r_maa", "g_maa", "w_maa"]
                        fidx = {"w_maa": 0, "k_maa": 1, "v_maa": 2,
                                "r_maa": 3, "g_maa": 4}
                        projs = {}
                        projw = {"r": w_r, "k": w_k, "v": w_v, "g": w_g}
                        pacts = {"r": Act.Sigmoid, "k": None, "v": None,
                                 "g": "silu"}
                        bxw = None
                        for n in bnames:
                            f = fidx[n]
                            bxf = pb.tile([128, DT, W], bf, tag="bx")
                            poff = E * (f % 4)
                            fj = f // 4
                            for mt in range(DT):
                                ps = psA.tile([128, W], f32, tag="mm")
                                nc.tensor.matmul(
                                    ps, lhsT=tmw2[poff:poff + E, fj,
                                                  mt * 128:(mt + 1) * 128],
                                    rhs=t320[poff:poff + E, fj, :],
                                    start=True, stop=True,
                                    tile_position=(poff, 0))
                                s2 = pw.tile([128, W], bf, tag="s2")
                                nc.vector.scalar_tensor_tensor(
                                    out=s2, in0=ps, scalar=maat[n][:, mt:mt + 1],
                                    in1=sx[:, mt, :], op0=Alu.add, op1=Alu.mult)
                                nc.gpsimd.tensor_add(bxf[:, mt, :], s2,
                                                     xln[:, mt, :])
                            ch = n[0]
                            if ch == "w":
                                bxw = bxf
                                continue
                            # projection for this bx
                            act = pacts[ch]
                            wt = projw[ch]
                            out_t = pk.tile([128, CT, W], bf, tag=f"proj_{ch}")
                            for mt in range(CT):
                                ps = psA.tile([128, W], f32, tag="mm")
                                mm_chain(ps,
                                         lambda kt, wt=wt, mt=mt:
                                             wt[:, kt, mt * 128:(mt + 1) * 128],
                                         lambda kt, bxf=bxf: bxf[:, kt, :], DT)
                                if act is None:
                                    nc.vector.tensor_copy(out_t[:, mt, :], ps)
                                elif act == "silu":
                                    sgt = pw.tile([128, W], bf, tag="sgt")
                                    nc.scalar.activation(sgt, ps, Act.Sigmoid)
                                    nc.vector.tensor_mul(out_t[:, mt, :], sgt,
                                                         ps)
                                else:
                                    nc.scalar.activation(out_t[:, mt, :], ps,
                                                         act)
                            projs[ch] = out_t
                        nc.sync.dma_start(out=g_d[w], in_=projs["g"])

                        ps = psA.tile([128, W], f32, tag="mm")
                        mm_chain(ps[0:ED, :],
                                 lambda kt: tdw1[:, kt, :],
                                 lambda kt: bxw[:, kt, :], DT)
                        t64 = pw.tile([ED, W], bf, tag="t64")
                        nc.scalar.activation(t64, ps[0:ED, :], Act.Tanh)
                        wtotW = pw.tile([128, CT, NCH], f32, tag="wtotW")
                        rt_t = pk.tile([128, CT, W], bf, tag="rt_t")
                        kt_t = pk.tile([128, CT, W], bf, tag="kt_t")
                        kh_t = pk.tile([128, CT, W], bf, tag="kh_t")
                        rk_t = pk.tile([128, CT, W], bf, tag="rk_t")
                        for mt in range(CT):
                            ps2 = psA.tile([128, W], f32, tag="mm")
                            nc.tensor.matmul(ps2,
                                             lhsT=tdw2[:, mt * 128:(mt + 1) * 128],
                                             rhs=t64, start=True, stop=True)
                            e_mt = pw.tile([128, W], f32, tag="e_mt")
                            nc.scalar.activation(e_mt, ps2, Act.Exp,
                                                 bias=tdb[:, mt:mt + 1],
                                                 scale=1.0)
                            P_mt = pw.tile([128, W], f32, tag="P_mt")
                            for c in range(NCH):
                                csl = slice(c * CKe, (c + 1) * CKe)
                                nc.vector.tensor_tensor_scan(
                                    out=P_mt[:, csl], data0=e_mt[:, csl],
                                    data1=zerosCK, initial=0.0,
                                    op0=Alu.add, op1=Alu.add)
                            Pex = pw.tile([128, W], f32, tag="Pex")
                            nc.vector.tensor_sub(Pex, P_mt, e_mt)
                            expA = pw.tile([128, W], f32, tag="expA")
                            nc.scalar.activation(expA, Pex, Act.Exp, scale=-1.0)
                            expB = pw.tile([128, W], f32, tag="expB")
                            nc.scalar.activation(expB, P_mt, Act.Exp, scale=1.0)
                            pv = P_mt.rearrange("p (c u) -> p c u", u=CKe)
                            nc.scalar.activation(
                                wtotW[:, mt, :].rearrange("p (c u) -> p c u",
                                                          u=1),
                                pv[:, :, CKe - 1:CKe], Act.Exp, scale=-1.0)
                            nc.vector.tensor_mul(rt_t[:, mt, :],
                                                 projs["r"][:, mt, :], expA)
                            nc.vector.tensor_mul(kt_t[:, mt, :],
                                                 projs["k"][:, mt, :], expB)
                            nc.vector.scalar_tensor_tensor(
                                out=rk_t[:, mt, :], in0=projs["r"][:, mt, :],
                                scalar=ut[:, mt:mt + 1],
                                in1=projs["k"][:, mt, :],
                                op0=Alu.mult, op1=Alu.mult)
                            for c in range(NCH):
                                csl = slice(c * CKe, (c + 1) * CKe)
                                nc.vector.tensor_scalar_mul(
                                    kh_t[:, mt, csl], kt_t[:, mt, csl],
                                    wtotW[:, mt, c:c + 1])
                        nc.sync.dma_start(out=rt_d[w], in_=rt_t)

                        # Wcum bookkeeping (capture, then advance)
                        for c in range(NCH):
                            g_ = w * NCH + c
                            nc.vector.tensor_copy(
                                Wcum[:, :, g_:g_ + 1],
                                Wrun.rearrange("p (a b) -> p a b", b=1))
                            wr2 = pw.tile([128, CT], f32, tag="wr2")
                            nc.vector.tensor_mul(
                                wr2.rearrange("p (a b) -> p a b", b=1),
                                Wrun.rearrange("p (a b) -> p a b", b=1),
                                wtotW[:, :, c:c + 1])
                            nc.vector.tensor_copy(Wrun, wr2)

                        ygT = None
                        if w == NW - 1:
                            ygT = pw.tile([128, CT, W], bf, tag="ygT",
                                          name="ygT")
                        for c in range(NCH):
                            csl = slice(c * CKe, (c + 1) * CKe)
                            for hg in range(NHG):
                                jts = [JG * hg + jl for jl in range(JG)]
                                VT = pw.tile([128, 512], bf, tag="VT")
                                KhT = pw.tile([128, 512], bf, tag="KhT")
                                for jl, jt in enumerate(jts):
                                    pt = psT.tile([128, 128], bf, tag="tr")
                                    nc.tensor.transpose(
                                        pt, projs["v"][:, jt, csl], ident)
                                    nc.vector.tensor_copy(
                                        VT[:, 128 * jl:128 * (jl + 1)], pt)
                                    pt2 = psT.tile([128, 128], bf, tag="tr")
                                    nc.tensor.transpose(pt2, kh_t[:, jt, csl],
                                                        ident)
                                    nc.vector.tensor_copy(
                                        KhT[:, 128 * jl:128 * (jl + 1)], pt2)

                                # d_i = sum_k r*u*k per head -> dT [tok, 4, 2]
                                ps_d = psB.tile([128, 512], f32, tag="wkvE")
                                for jl, jt in enumerate(jts):
                                    nc.tensor.matmul(
                                        ps_d[0:2, 128 * jl:128 * (jl + 1)],
                                        lhsT=blkdiag, rhs=rk_t[:, jt, csl],
                                        start=True, stop=True)
                                d8 = pw.tile([128, CKe], bf, tag="d8")
                                for jl in range(JG):
                                    nc.vector.tensor_copy(
                                        d8[32 * jl:32 * jl + 2, :],
                                        ps_d[0:2, 128 * jl:128 * (jl + 1)])
                                ptd = psT.tile([128, 128], bf, tag="tr")
                                nc.tensor.transpose(ptd, d8, ident)
                                dT = pw.tile([128, JG, 2], bf, tag="dT")
                                nc.vector.tensor_copy(
                                    dT, ptd.rearrange("p (a b) -> p a b",
                                                      b=32)[:, :, 0:2])

                                psO = psB.tile([128, 512], f32, tag="wkvO")
                                psE = psB.tile([128, 512], f32, tag="wkvE")
                                psD = psB.tile([128, 512], f32, tag="wkvD")
                                Am = pw.tile([128, 512], bf, tag="Am")
                                Am2 = pw.tile([128, 512], bf, tag="Am2")
                                for hl in range(8):
                                    poff = HS * (hl % 2)
                                    jt = jts[hl // 2]
                                    bank = psE if hl % 2 == 0 else psD
                                    nc.tensor.matmul(
                                        bank[:, 128 * (hl // 2):
                                             128 * (hl // 2 + 1)],
                                        lhsT=kt_t[poff:poff + HS, jt, csl],
                                        rhs=rt_t[poff:poff + HS, jt, csl],
                                        start=True, stop=True,
                                        tile_position=(poff, 0))
                                for hl in range(8):
                                    bank = psE if hl % 2 == 0 else psD
                                    am = Am if hl % 2 == 0 else Am2
                                    nc.vector.tensor_mul(
                                        am[:, 128 * (hl // 2):
                                           128 * (hl // 2 + 1)],
                                        bank[:, 128 * (hl // 2):
                                             128 * (hl // 2 + 1)],
                                        maskstr[0:CKe, 0:CKe])
                                for hl in range(8):
                                    poff = HS * (hl % 2)
                                    jt = jts[hl // 2]
                                    am = Am if hl % 2 == 0 else Am2
                                    nc.tensor.matmul(
                                        psO[:, HS * hl:HS * (hl + 1)],
                                        lhsT=am[:, 128 * (hl // 2):
                                                128 * (hl // 2 + 1)],
                                        rhs=VT[:, HS * hl:HS * (hl + 1)],
                                        start=True, stop=False)
                                    nc.tensor.matmul(
                                        psO[:, HS * hl:HS * (hl + 1)],
                                        lhsT=rt_t[poff:poff + HS, jt, csl],
                                        rhs=S_cur[poff:poff + HS, jt, :],
                                        start=False, stop=True,
                                        tile_position=(poff, 0))

                                O_sb = pw.tile([128, 512], bf, tag="O_sb")
                                for hl in range(8):
                                    nc.vector.scalar_tensor_tensor(
                                        out=O_sb[:, HS * hl:HS * (hl + 1)],
                                        in0=VT[:, HS * hl:HS * (hl + 1)],
                                        scalar=dT[:, hl // 2,
                                                  (hl % 2):(hl % 2) + 1],
                                        in1=psO[:, HS * hl:HS * (hl + 1)],
                                        op0=Alu.mult, op1=Alu.add)
                                nc.sync.dma_start(out=opre_d[w, c, hg],
                                                  in_=O_sb[0:CKe, :])

                                psSe = psB.tile([128, JG, HS], f32, tag="wkvSE")
                                psSd = psB.tile([128, JG, HS], f32, tag="wkvSD")
                                for hl in range(8):
                                    poff = HS * (hl % 2)
                                    jl = hl // 2
                                    bank = psSe if hl % 2 == 0 else psSd
                                    nc.tensor.matmul(
                                        bank[poff:poff + HS, jl, :],
                                        lhsT=KhT[:, HS * hl:HS * (hl + 1)],
                                        rhs=VT[:, HS * hl:HS * (hl + 1)],
                                        start=True, stop=True,
                                        tile_position=(0, poff))
                                S_new = sp.tile([128, CT, HS], bf, tag="S")
                                if hg == 0:
                                    # copy untouched half first
                                    nc.gpsimd.tensor_copy(
                                        S_new[:, JG:CT, :], S_cur[:, JG:CT, :])
                                else:
                                    nc.gpsimd.tensor_copy(
                                        S_new[:, 0:JG, :], S_cur[:, 0:JG, :])
                                for jl, jt in enumerate(jts):
                                    nc.vector.scalar_tensor_tensor(
                                        out=S_new[0:HS, jt, :],
                                        in0=S_cur[0:HS, jt, :],
                                        scalar=wtotW[0:HS, jt, c:c + 1],
                                        in1=psSe[0:HS, jl, :],
                                        op0=Alu.mult, op1=Alu.add)
                                    nc.vector.scalar_tensor_tensor(
                                        out=S_new[HS:128, jt, :],
                                        in0=S_cur[HS:128, jt, :],
                                        scalar=wtotW[HS:128, jt, c:c + 1],
                                        in1=psSd[HS:128, jl, :],
                                        op0=Alu.mult, op1=Alu.add)
                                S_cur = S_new

                                if w == NW - 1:
                                    # provisional GN/gate for payload resid
                                    Ogn = group_norm(pw, O_sb, gnw_r, gnb_r,
                                                     hg)
                                    for jl, jt in enumerate(jts):
                                        pt3 = psT.tile([128, 128], bf,
                                                       tag="tr")
                                        nc.tensor.transpose(
                                            pt3,
                                            Ogn[:, 128 * jl:128 * (jl + 1)],
                                            ident)
                                        nc.vector.tensor_mul(
                                            ygT[:, jt, csl], pt3,
                                            projs["g"][:, jt, csl])

                        if w == NW - 1:
                            # provisional Wo + last-token residual -> payload
                            paysb = px.tile([128, PH], f32, tag="paysb")
                            S32 = px.tile([128, CT, HS], f32, tag="S32")
                            nc.vector.tensor_copy(S32, S_cur)
                            nc.vector.tensor_copy(
                                paysb[:, 0:PAYS],
                                S32.rearrange("p a b -> p (a b)"))
                            for mt in range(DT):
                                ps = psA.tile([128, W], f32, tag="mm")
                                mm_chain(ps,
                                         lambda kt, mt=mt:
                                             wo[:, kt, mt * 128:(mt + 1) * 128],
                                         lambda kt: ygT[:, kt, :], CT)
                                nc.vector.tensor_add(
                                    paysb[:, PAYS + mt:PAYS + mt + 1],
                                    xw[:, mt, W - 1:W], ps[:, W - 1:W])
                            # zero h=1's payload; AllReduce-add then hands
                            # h=0's state+resid to both cores of the pair
                            paym = px.tile([128, PH], f32, tag="paym")
                            nc.vector.tensor_scalar_mul(paym, paysb,
                                                        itmask_t[:, 0:1])
                            nc.sync.dma_start(out=pay_t, in_=paym)
                            if _STOP >= 1:
                                nc.gpsimd.collective_compute(
                                    "AllReduce", Alu.add,
                                    replica_groups=REPLICA_GROUPS,
                                    ins=[pay_t.opt()], outs=[payg_t.opt()])
                            else:
                                nc.sync.dma_start(out=payg_t, in_=pay_t)

                # ------------- time mix: correction pass -------------
                if _STOP < 2:
                    continue
                with tc.tile_pool(name=f"cor{l}", bufs=2) as pc, \
                     tc.tile_pool(name=f"corw{l}", bufs=1) as pcw, \
                     tc.tile_pool(name=f"wtmB2{l}", bufs=1) as wpB2, \
                     tc.tile_pool(name=f"vecc{l}", bufs=1) as vpc, \
                     tc.tile_pool(name=f"psc{l}", bufs=2, space="PSUM") as psC, \
                     tc.tile_pool(name=f"psd{l}", bufs=1, space="PSUM") as psD2, \
                     tc.tile_pool(name=f"pse{l}", bufs=1, space="PSUM") as psT2:

                    wo2 = wpB2.tile([128, CT, D], bf, tag="wo2")
                    for j in range(CT):
                        nc.sync.dma_start(out=wo2[:, j, :], in_=Wo_d[l, j])
                    gnw_r2 = vpc.tile([128, CH], bf, tag="gnw_r2")
                    gnb_r2 = vpc.tile([128, CH], bf, tag="gnb_r2")
                    if not skip_gn_affine:
                        nc.sync.dma_start(out=gnw_r2, in_=bcast_ap(gnw_d[l]))
                        nc.sync.dma_start(out=gnb_r2, in_=bcast_ap(gnb_d[l]))

                    S_in = vpc.tile([128, CT, HS], f32, tag="S_in")
                    nc.sync.dma_start(
                        out=S_in.rearrange("p a b -> p (a b)"),
                        in_=payg_t[:, 0:PAYS])
                    # next-phase boundary resid (masked)
                    bres_raw = vpc.tile([128, DT], f32, tag="bres_raw")
                    nc.sync.dma_start(out=bres_raw,
                                      in_=payg_t[:, PAYS:PH])
                    nc.vector.tensor_scalar_mul(bres, bres_raw,
                                                tmask_t[:, 0:1])

                    for w in (range(NW) if _SUB >= 2 else []):
                        sl = slice(w * W, (w + 1) * W)
                        xwc = pc.tile([128, DT, W], f32, tag="xwc")
                        for j in range(DT):
                            nc.sync.dma_start(out=xwc[:, j, :],
                                              in_=x_src(phase_tm, j, sl))
                        rt_sb = pc.tile([128, CT, W], bf, tag="rt_sb")
                        nc.sync.dma_start(out=rt_sb, in_=rt_d[w])
                        g_sb = pc.tile([128, CT, W], bf, tag="g_sb")
                        nc.sync.dma_start(out=g_sb, in_=g_d[w])
                        ygT2 = pcw.tile([128, CT, W], bf, tag="ygT2")
                        for c in range(NCH):
                            csl = slice(c * CKe, (c + 1) * CKe)
                            g_ = w * NCH + c
                            S_corr = pcw.tile([128, CT, HS], bf, tag="S_corr")
                            if _SUB >= 3:
                                for jt in range(CT):
                                    nc.vector.tensor_scalar_mul(
                                        S_corr[:, jt, :], S_in[:, jt, :],
                                        Wcum[:, jt, g_:g_ + 1])
                            for hg in range(NHG):
                                jts = [JG * hg + jl for jl in range(JG)]
                                opre_sb = pcw.tile([128, 512], bf, tag="opre_sb")
                                nc.sync.dma_start(out=opre_sb[0:CKe, :],
                                                  in_=opre_d[w, c, hg])
                                O_c = pcw.tile([128, 512], f32, tag="O_c")
                                if _SUB >= 4:
                                    psO2e = psD2.tile([128, 512], f32,
                                                      tag="corrE",
                                                      name="psO2e")
                                    psO2d = psD2.tile([128, 512], f32,
                                                      tag="corrD",
                                                      name="psO2d")
                                    for hl in range(8):
                                        poff = HS * (hl % 2)
                                        jt = jts[hl // 2]
                                        bank = psO2e if hl % 2 == 0 else psO2d
                                        nc.tensor.matmul(
                                            bank[:, HS * hl:HS * (hl + 1)],
                                            lhsT=rt_sb[poff:poff + HS, jt, csl],
                                            rhs=S_corr[poff:poff + HS, jt, :],
                                            start=True, stop=True,
                                            tile_position=(poff, 0))
                                if _SUB >= 5:
                                    for hl in range(8):
                                        s = slice(HS * hl, HS * (hl + 1))
                                        bank = psO2e if hl % 2 == 0 else psO2d
                                        nc.vector.tensor_add(
                                            O_c[0:CKe, s], opre_sb[0:CKe, s],
                                            bank[0:CKe, s])
                                else:
                                    nc.vector.tensor_copy(O_c[0:CKe, :],
                                                          opre_sb[0:CKe, :])
                                Ogn = group_norm(pcw, O_c, gnw_r2, gnb_r2, hg)
                                for jl, jt in enumerate(jts):
                                    pt3 = psT2.tile([128, 128], bf, tag="tr2")
                                    nc.tensor.transpose(
                                        pt3, Ogn[:, 128 * jl:128 * (jl + 1)],
                                        ident)
                                    nc.vector.tensor_mul(
                                        ygT2[:, jt, csl], pt3,
                                        g_sb[:, jt, csl])
                        for mt in range(DT):
                            ps = psC.tile([128, W], f32, tag="mmc")
                            mm_chain(ps,
                                     lambda kt, mt=mt:
                                         wo2[:, kt, mt * 128:(mt + 1) * 128],
                                     lambda kt: ygT2[:, kt, :], CT)
                            nc.vector.tensor_add(xwc[:, mt, :], xwc[:, mt, :],
                                                 ps)
                            nc.sync.dma_start(out=x_dst(phase_tm, mt, sl),
                                              in_=xwc[:, mt, :])

                # ---------------- channel mix ----------------
                if _STOP < 3:
                    continue
                with tc.tile_pool(name=f"wcm{l}", bufs=1) as wp2, \
                     tc.tile_pool(name=f"vc2{l}", bufs=1) as vp2, \
                     tc.tile_pool(name=f"cmx{l}", bufs=1) as px2, \
                     tc.tile_pool(name=f"cml{l}", bufs=2) as pl2, \
                     tc.tile_pool(name=f"cma{l}", bufs=1) as pw2, \
                     tc.tile_pool(name=f"psf{l}", bufs=4, space="PSUM") as psC2:

                    cwk = wp2.tile([128, DT, FT * 128], bf, tag="cwk")
                    cwr = wp2.tile([128, DT, D], bf, tag="cwr")
                    for j in range(DT):
                        nc.sync.dma_start(out=cwk[:, j, :], in_=cWk_d[l, j])
                        nc.sync.dma_start(out=cwr[:, j, :], in_=cWr_d[l, j])
                    cwv = wp2.tile([128, FT, D], bf, tag="cwv")
                    for j in range(FT):
                        nc.sync.dma_start(out=cwv[:, j, :], in_=cWv_d[l, j])
                    ln2w = vp2.tile([128, DT], f32, tag="ln2w")
                    nc.sync.dma_start(out=ln2w, in_=ln2_d[l])
                    ckm = vp2.tile([128, DT], f32, tag="ckm")
                    nc.sync.dma_start(out=ckm, in_=maas_d["ck_maa"][l])
                    crm = vp2.tile([128, DT], f32, tag="crm")
                    nc.sync.dma_start(out=crm, in_=maas_d["cr_maa"][l])

                    bres_v2 = px2.tile([128, DT, 1], f32, tag="bres_v2")
                    nc.vector.tensor_copy(
                        bres_v2.rearrange("p a b -> p (a b)"), bres)
                    bln2 = layer_norm(px2, psC2, ln2w, bres_v2, 1, tag="bln2")

                    xln2_prev = None
                    for w in range(NW):
                        sl = slice(w * W, (w + 1) * W)
                        xw2 = pw2.tile([128, DT, W], f32, tag="xw2")
                        for j in range(DT):
                            nc.sync.dma_start(out=xw2[:, j, :],
                                              in_=x_src(phase_cm, j, sl))
                        xln2 = pl2.tile([128, DT, W], bf, tag="xln2")
                        layer_norm(pw2, psC2, ln2w, xw2, W, out=xln2)
                        prev_last = (bln2 if w == 0
                                     else xln2_prev[:, :, W - 1:W])
                        sx2 = token_shift(pw2, xln2, prev_last, W)
                        xln2_prev = xln2
                        kx = pw2.tile([128, DT, W], bf, tag="kx")
                        rx = pw2.tile([128, DT, W], bf, tag="rx")
                        for j in range(DT):
                            nc.vector.scalar_tensor_tensor(
                                out=kx[:, j, :], in0=sx2[:, j, :],
                                scalar=ckm[:, j:j + 1], in1=xln2[:, j, :],
                                op0=Alu.mult, op1=Alu.add)
                            nc.vector.scalar_tensor_tensor(
                                out=rx[:, j, :], in0=sx2[:, j, :],
                                scalar=crm[:, j:j + 1], in1=xln2[:, j, :],
                                op0=Alu.mult, op1=Alu.add)

                        kk = pw2.tile([128, FT, W], bf, tag="kk")
                        for mt in range(FT):
                            ps = psC2.tile([128, W], f32, tag="mm")
                            mm_chain(ps,
                                     lambda kt, mt=mt:
                                         cwk[:, kt, mt * 128:(mt + 1) * 128],
                                     lambda kt: kx[:, kt, :], DT)
                            kk0 = pw2.tile([128, W], bf, tag="kk0")
                            nc.vector.tensor_copy(kk0, ps)
                            nc.vector.scalar_tensor_tensor(
                                out=kk[:, mt, :], in0=kk0, scalar=0.0, in1=kk0,
                                op0=Alu.max, op1=Alu.mult)

                        rr = pw2.tile([128, DT, W], bf, tag="rr")
                        for mt in range(DT):
                            ps = psC2.tile([128, W], f32, tag="mm")
                            mm_chain(ps,
                                     lambda kt, mt=mt:
                                         cwr[:, kt, mt * 128:(mt + 1) * 128],
                                     lambda kt: rx[:, kt, :], DT)
                            nc.scalar.activation(rr[:, mt, :], ps, Act.Sigmoid)

                        tmp2 = pw2.tile([128, W], f32, tag="tmp2")
                        for mt in range(DT):
                            ps = psC2.tile([128, W], f32, tag="mm")
                            mm_chain(ps,
                                     lambda kt, mt=mt:
                                         cwv[:, kt, mt * 128:(mt + 1) * 128],
                                     lambda kt: kk[:, kt, :], FT)
                            nc.vector.tensor_mul(tmp2, rr[:, mt, :], ps)
                            nc.vector.tensor_add(xw2[:, mt, :], xw2[:, mt, :],
                                                 tmp2)
                            nc.sync.dma_start(out=x_dst(phase_cm, mt, sl),
                                              in_=xw2[:, mt, :])

                        if w == NW - 1 and l < L - 1:
                            # ship last-token residual for next layer's tm
                            pay2sb = px2.tile([128, DT], f32, tag="pay2sb")
                            nc.vector.tensor_scalar_mul(
                                pay2sb,
                                xw2[:, :, W - 1:W].rearrange(
                                    "p a b -> p (a b)"),
                                itmask_t[:, 0:1])
                            nc.sync.dma_start(out=pay2_t, in_=pay2sb)
                            nc.gpsimd.collective_compute(
                                "AllReduce", Alu.add,
                                replica_groups=REPLICA_GROUPS,
                                ins=[pay2_t.opt()], outs=[payg2_t.opt()])
                    if l < L - 1:
                        with tc.tile_pool(name=f"bnd{l}", bufs=1) as pbn:
                            braw2 = pbn.tile([128, DT], f32, tag="braw2")
                            nc.sync.dma_start(out=braw2, in_=payg2_t)
                            nc.vector.tensor_scalar_mul(bres, braw2,
                                                        tmask_t[:, 0:1])

            # ---------------- final LN ----------------
            with tc.tile_pool(name="fin", bufs=2) as pf, \
                 tc.tile_pool(name="psfin", bufs=2, space="PSUM") as psF, \
                 tc.tile_pool(name="vecf", bufs=1) as vf:
                lnfw = vf.tile([128, DT], f32, tag="lnfw")
                nc.sync.dma_start(out=lnfw, in_=lnf_d[:, :])
                for w in range(NW):
                    sl = slice(w * W, (w + 1) * W)
                    xw3 = pf.tile([128, DT, W], f32, tag="xw3")
                    for j in range(DT):
                        nc.sync.dma_start(out=xw3[:, j, :],
                                          in_=x_src(2 * L, j, sl))
                    yw = layer_norm(pf, psF, lnfw, xw3, W, out_dt=f32,
                                    tag="fln")
                    for j in range(DT):
                        nc.sync.dma_start(out=y_d[128 * j:128 * (j + 1), sl],
                                          in_=yw[:, j, :])
    if split_waits:
        _split_sync_waits(nc, scratch=wsplit_sem)
    return nc


# ===================== host side =====================

_CACHE = {}


def _prep_core_inputs(inputs, core, T):
    b, half = core // 2, core % 2
    TH = T // 2
    tsl = slice(half * TH, (half + 1) * TH)

    def kt_tiles(w):
        return np.ascontiguousarray(
            w.reshape(w.shape[0] // 128, 128, w.shape[1]).astype(BF16))

    def vec_tiles(v):
        return np.ascontiguousarray(v.reshape(-1, 128).T.astype(np.float32))

    out = {"x": np.ascontiguousarray(
        inputs["x"][b][tsl].T.astype(np.float32))}
    if half == 1:
        out["xb0"] = vec_tiles(inputs["x"][b][TH - 1])
        out["tmask"] = np.ones((1, CT), np.float32)
    else:
        out["xb0"] = np.zeros((128, DT), np.float32)
        out["tmask"] = np.zeros((1, CT), np.float32)
    out["Wr"] = np.stack([kt_tiles(inputs["Wr"][l]) for l in range(L)])
    out["Wk"] = np.stack([kt_tiles(inputs["Wk"][l]) for l in range(L)])
    out["Wv"] = np.stack([kt_tiles(inputs["Wv"][l]) for l in range(L)])
    out["Wg"] = np.stack([kt_tiles(inputs["Wg"][l]) for l in range(L)])
    out["tm_w1"] = np.stack([kt_tiles(inputs["tm_w1"][l]) for l in range(L)])
    tw2 = np.zeros((L, 2, 128, D), BF16)
    for l in range(L):
        for f in range(5):
            tw2[l, f // 4, E * (f % 4):E * (f % 4) + E] = \
                inputs["tm_w2"][l, f].astype(BF16)
    out["tm_w2"] = tw2
    out["td_w1"] = np.stack([kt_tiles(inputs["td_w1"][l]) for l in range(L)])
    out["td_w2"] = np.stack([inputs["td_w2"][l].astype(BF16)
                             for l in range(L)])
    out["Wo"] = np.stack([kt_tiles(inputs["Wo"][l]) for l in range(L)])
    out["cWk"] = np.stack([kt_tiles(inputs["cWk"][l]) for l in range(L)])
    out["cWv"] = np.stack([kt_tiles(inputs["cWv"][l]) for l in range(L)])
    out["cWr"] = np.stack([kt_tiles(inputs["cWr"][l]) for l in range(L)])

    out["ln1_w"] = np.stack([vec_tiles(inputs["ln1_w"][l]) for l in range(L)])
    out["ln2_w"] = np.stack([vec_tiles(inputs["ln2_w"][l]) for l in range(L)])
    out["ln_out_w"] = vec_tiles(inputs["ln_out_w"])
    for n in ["x_maa", "w_maa", "k_maa", "v_maa", "r_maa", "g_maa",
              "ck_maa", "cr_maa"]:
        out[n] = np.stack([vec_tiles(inputs[n][l]) for l in range(L)])
    out["td_bias"] = np.stack(
        [vec_tiles(inputs["time_decay"][l].reshape(-1)) for l in range(L)])
    out["u_first"] = np.stack(
        [vec_tiles(inputs["time_first"][l].reshape(-1)) for l in range(L)])
    out["gn_w"] = np.stack([inputs["gn_w"][l].astype(np.float32)
                            for l in range(L)])
    out["gn_b"] = np.stack([inputs["gn_b"][l].astype(np.float32)
                            for l in range(L)])
    out["ident_c"] = np.eye(128, dtype=BF16)
    out["maskstr_c"] = np.triu(np.ones((128, 128), np.float32), 1)
    bd = np.zeros((128, 2), BF16)
    bd[0:64, 0] = 1
    bd[64:128, 1] = 1
    out["blkdiag_c"] = bd
    return out


def _get_nc(T, skip_gn_affine, skip_ln_w):
    key = (T, skip_gn_affine, skip_ln_w)
    if key not in _CACHE:
        nc = bass.Bass(trn_type="TRN2", num_devices=8)
        build(nc, T // 2, skip_gn_affine, skip_ln_w)
        _CACHE[key] = nc
    return _CACHE[key]


def _flags(inputs):
    skip_gn = bool(np.all(inputs["gn_w"] == 1.0)
                   and np.all(inputs["gn_b"] == 0.0))
    skip_ln = bool(np.all(inputs["ln1_w"] == 1.0)
                   and np.all(inputs["ln2_w"] == 1.0)
                   and np.all(inputs["ln_out_w"] == 1.0))
    return skip_gn, skip_ln


def kernel(**inputs):
    inputs = {k: np.asarray(v) for k, v in inputs.items()}
    T = inputs["x"].shape[1]
    skip_gn, skip_ln = _flags(inputs)
    nc = _get_nc(T, skip_gn, skip_ln)
    in_maps = [_prep_core_inputs(inputs, c, T) for c in range(8)]
    res = run_bass_kernel_spmd(nc, in_maps, core_ids=list(range(8)))
    TH = T // 2
    outs = []
    for b in range(B):
        y = np.empty((T, D), np.float32)
        y[:TH] = res.results[2 * b]["y"].T
        y[TH:] = res.results[2 * b + 1]["y"].T
        outs.append(y)
    return np.stack(outs).astype(np.float32)
